# revision 7
# baseline (speedup 1.0000x reference)
"""Bi-directional Mamba block (concat variant) on 8 Trainium2 NeuronCores.

Sharding: core = (direction g in {0,1}) x (batch b in {0,1}) x (d_inner half dh in {0,1}).
Each core runs one direction's Mamba for one batch element over its local 512 of
the 1024 d_inner channels.  The causal depthwise conv is folded into the input
projection as 4 time-shifted matmuls (host pre-merges conv_w into in_w).  The
x-projection contracts over all of d_inner, so the two cores of a (g,b) pair
AllReduce their [64, 512] partial per time chunk.  out_proj partials (contraction
over local channels) are summed on the host during unsharding.

Device layout is [channel-partition, time-free].  The SSM scan uses the hardware
tensor_tensor_scan (VectorE) over 1024-wide time spans: per (d-block of 128,
state n of 16), ScalarE computes dA = exp(delta * A[:,n]) with A as per-partition
activation scale, VectorE forms dBu = (delta*xc) * B_n and C*h in bf16 (2x DVE
mode), and the 16 state planes are summed by PE identity-matmuls into PSUM.
B_n/C_n rows are broadcast across partitions with selector matmuls on the PE.
"""

import os
import sys

sys.path.insert(0, "/opt/trn_rl_repo")

import numpy as np
import ml_dtypes
import concourse.bacc as bacc
import concourse.mybir as mybir
import concourse.tile as tile
from concourse.bass_utils import run_bass_kernel_spmd

F32 = mybir.dt.float32
F32R = mybir.dt.float32r
BF16 = mybir.dt.bfloat16
AF = mybir.ActivationFunctionType
OP = mybir.AluOpType

T = 2048          # sequence length
DM = 512          # per-direction d_model
DI = 1024         # full d_inner
DL = 512          # local d_inner channels per core
DS = 16           # d_state
RK = 32           # dt_rank
KW = 4            # d_conv
TC = 512          # time chunk (stage B / PSUM granularity)
SC = 1024         # scan span (two time chunks)
NTP = T // SC     # 2 tc-pairs
NKC = DM // 128   # 4 contraction chunks for in_proj
NBLK = DL // 128  # 4 local channel blocks
NOB = DM // 128   # 4 output blocks

GROUPS = [[0, 1], [2, 3], [4, 5], [6, 7]]

LAST_EXEC_NS = None
LAST_RESULTS = None


def round_f32r(x):
    """Round fp32 to fp32r (11-bit mantissa, round-to-nearest-even)."""
    u = np.ascontiguousarray(x, np.float32).view(np.uint32)
    lsb = (u >> 12) & np.uint32(1)
    ur = (u + np.uint32(0x7FF) + lsb) & np.uint32(0xFFFFF000)
    return ur.view(np.float32)


def _build_program():
    nc = bacc.Bacc("TRN2", target_bir_lowering=False, debug=False, num_devices=8)

    d = lambda name, shape: nc.dram_tensor(name, shape, F32, kind="ExternalInput").ap()
    dr = lambda name, shape: nc.dram_tensor(name, shape, F32R, kind="ExternalInput").ap()
    xt = dr("xt", [128, NKC * (T + 3)])         # x dir-half, transposed, 3-col zero pad, kc-major
    wcin = dr("wcin", [128, KW * NKC * DL])     # conv-fused in_proj lhsT, (k,kc)-major
    wz = dr("wz", [128, NKC * DL])              # z in_proj lhsT, kc-major
    bconv = d("bconv", [128, NBLK])
    wxp = d("wxp", [128, NBLK * 64])            # xproj lhsT (local), kc-major; fp32 mm
    wdt = dr("wdt", [32, DL])                   # dt_proj lhsT
    bdt = d("bdt", [128, NBLK])
    alog = d("alog", [128, NBLK * DS])
    dvec = d("dvec", [128, NBLK])
    wout = dr("wout", [128, NBLK * DM])         # out_proj lhsT, dblk-major
    idenb = nc.dram_tensor("idenb", [128, 128], BF16, kind="ExternalInput").ap()
    selb = dr("selb", [64, DS * 128])           # selector: dbc row 32+n -> 128 cols
    selc = dr("selc", [64, DS * 128])           # selector: dbc row 48+n
    outp = nc.dram_tensor("outp", [128, NOB * T], F32, kind="ExternalOutput").ap()

    with tile.TileContext(nc) as tc_:
        _body(tc_, nc, xt, wcin, wz, bconv, wxp, wdt, bdt, alog, dvec, wout,
              idenb, selb, selc, outp)
    nc.compile()
    return nc


def _body(tc_, nc, xt, wcin, wz, bconv, wxp, wdt, bdt, alog, dvec, wout,
          idenb, selb, selc, outp):
    from contextlib import ExitStack
    ctx = ExitStack()
    with ctx:
        wp = ctx.enter_context(tc_.tile_pool(name="wp", bufs=1))
        xtp = ctx.enter_context(tc_.tile_pool(name="xtp", bufs=5))
        wcp = ctx.enter_context(tc_.tile_pool(name="wcp", bufs=4))
        seq = ctx.enter_context(tc_.tile_pool(name="seq", bufs=2))
        sq1 = ctx.enter_context(tc_.tile_pool(name="sq1", bufs=1))
        scp = ctx.enter_context(tc_.tile_pool(name="scp", bufs=2))
        bcp = ctx.enter_context(tc_.tile_pool(name="bcp", bufs=2))
        gp = ctx.enter_context(tc_.tile_pool(name="gp", bufs=2))
        ygp = ctx.enter_context(tc_.tile_pool(name="ygp", bufs=8))
        osp = ctx.enter_context(tc_.tile_pool(name="osp", bufs=2))
        drp = ctx.enter_context(tc_.tile_pool(name="drp", bufs=2, space="DRAM"))
        pm = ctx.enter_context(tc_.tile_pool(name="pm", bufs=2, space="PSUM"))
        pbc = ctx.enter_context(tc_.tile_pool(name="pbc", bufs=1, space="PSUM"))
        pyp = ctx.enter_context(tc_.tile_pool(name="pyp", bufs=1, space="PSUM"))

        # ---- persistent weights ----
        def wtile(name, shape, src, dt_=F32):
            t_ = wp.tile(shape, dt_, tag=name, name=name)
            nc.sync.dma_start(t_[:], src[:])
            return t_

        wz_sb = wtile("wz", [128, NKC * DL], wz, F32R)
        wxp_sb = wtile("wxp", [128, NBLK * 64], wxp)
        wdt_sb = wtile("wdt", [32, DL], wdt, F32R)
        bdt_sb = wtile("bdt", [128, NBLK], bdt)
        bconv_sb = wtile("bconv", [128, NBLK], bconv)
        alog_sb = wtile("alog", [128, NBLK * DS], alog)
        dvec_sb = wtile("dvec", [128, NBLK], dvec)
        wout_sb = wtile("wout", [128, NBLK * DM], wout, F32R)
        idenb_sb = wtile("idenb", [128, 128], idenb, BF16)
        selb_sb = wtile("selb", [64, DS * 128], selb, F32R)
        selc_sb = wtile("selc", [64, DS * 128], selc, F32R)

        # A = -exp(A_log)
        a_tmp = wp.tile([128, NBLK * DS], F32, tag="a_tmp")
        nc.scalar.activation(a_tmp[:], alog_sb[:], AF.Exp)
        a_sb = wp.tile([128, NBLK * DS], F32, tag="a_sb")
        nc.vector.tensor_scalar_mul(a_sb[:], a_tmp[:], -1.0)

        # scan state [128, blk*16+n], init 0
        state = wp.tile([128, NBLK * DS], F32, tag="state")
        nc.vector.memset(state[:], 0.0)

        for tp in range(NTP):
            xcl = sq1.tile([128, NBLK * SC], F32, tag="xcl")
            zsil = sq1.tile([128, NBLK * SC], F32, tag="zsil")
            delta = seq.tile([128, NBLK * SC], F32, tag="delta")
            dbcrs = []
            for hf in range(2):
                t = tp * 2 + hf
                # ---- stage B ----
                xts = []
                for kc in range(NKC):
                    xtile = xtp.tile([128, TC + 3], F32R, tag="xts", name="xtile")
                    nc.sync.dma_start(xtile[:], xt[:, kc * (T + 3) + t * TC:
                                                   kc * (T + 3) + t * TC + TC + 3])
                    xts.append(xtile)

                # conv-fused in_proj, streaming weights, 2 channel blocks at a time
                for mp in range(NBLK // 2):
                    pss = [pm.tile([128, TC], F32, tag="mm", name="psin")
                           for _ in range(2)]
                    for k in range(KW):
                        for kc in range(NKC):
                            wtl = wcp.tile([128, DL], F32R, tag="wcin", name="wtl")
                            nc.sync.dma_start(
                                wtl[:], wcin[:, (k * NKC + kc) * DL:
                                             (k * NKC + kc) * DL + DL])
                            for i in range(2):
                                mb = mp * 2 + i
                                nc.tensor.matmul(
                                    pss[i][:], wtl[:, mb * 128:(mb + 1) * 128],
                                    xts[kc][:, k:k + TC],
                                    start=(k == 0 and kc == 0),
                                    stop=(k == KW - 1 and kc == NKC - 1))
                    for i in range(2):
                        mb = mp * 2 + i
                        nc.scalar.activation(
                            xcl[:, mb * SC + hf * TC:mb * SC + hf * TC + TC],
                            pss[i][:], AF.Silu, bias=bconv_sb[:, mb:mb + 1])

                # xproj partial (local half) -> AllReduce across the (g,b) pair
                psd = pm.tile([64, TC], F32, tag="mm", name="psd")
                for mb in range(NBLK):
                    nc.tensor.matmul(
                        psd[:], wxp_sb[:, mb * 64:(mb + 1) * 64],
                        xcl[:, mb * SC + hf * TC:mb * SC + hf * TC + TC],
                        start=(mb == 0), stop=(mb == NBLK - 1))
                dbp = gp.tile([64, TC], F32, tag="dbp", bufs=1)
                nc.scalar.copy(dbp[:], psd[:])
                dbi = drp.tile([64, TC], F32, tag="dbi")
                dbo = drp.tile([64, TC], F32, tag="dbo")
                nc.sync.dma_start(dbi[:], dbp[:])
                nc.gpsimd.collective_compute(
                    "AllReduce", OP.add, replica_groups=GROUPS,
                    ins=[dbi.opt()], outs=[dbo.opt()])
                dbc = gp.tile([64, TC], F32, tag="dbc", bufs=1)
                nc.sync.dma_start(dbc[:], dbo[:])
                dbcr = gp.tile([64, TC], F32R, tag="dbcr")
                nc.scalar.copy(dbcr[:], dbc[:])
                dbcrs.append(dbcr)

                # z branch (local half only)
                for zb in range(NBLK):
                    ps = pm.tile([128, TC], F32, tag="mm", name="psz")
                    for kc in range(NKC):
                        nc.tensor.matmul(
                            ps[:],
                            wz_sb[:, kc * DL + zb * 128:kc * DL + zb * 128 + 128],
                            xts[kc][:, 3:3 + TC],
                            start=(kc == 0), stop=(kc == NKC - 1))
                    nc.scalar.activation(zsil[:, zb * SC + hf * TC:
                                               zb * SC + hf * TC + TC], ps[:], AF.Silu)

                # delta = softplus(dt_proj + dt_b) = ln(1 + e^x), x clamped at 80
                for blk in range(NBLK):
                    ps = pm.tile([128, TC], F32, tag="mm", name="psdt")
                    nc.tensor.matmul(
                        ps[:], wdt_sb[:, blk * 128:(blk + 1) * 128],
                        dbcr[0:32, :], start=True, stop=True)
                    spt = scp.tile([128, TC], F32, tag="sptmp")
                    nc.vector.tensor_scalar(spt[:], ps[:], bdt_sb[:, blk:blk + 1],
                                            80.0, OP.add, OP.min)
                    spe = scp.tile([128, TC], F32, tag="spexp")
                    nc.scalar.activation(spe[:], spt[:], AF.Exp)
                    nc.scalar.activation(delta[:, blk * SC + hf * TC:
                                               blk * SC + hf * TC + TC],
                                         spe[:], AF.Ln, bias=1.0)

            # du = delta * xc_local (bf16 for the 2x DVE path)
            du = seq.tile([128, NBLK * SC], BF16, tag="du")
            for blk in range(NBLK):
                nc.vector.tensor_mul(du[:, blk * SC:(blk + 1) * SC],
                                     delta[:, blk * SC:(blk + 1) * SC],
                                     xcl[:, blk * SC:(blk + 1) * SC])

            # ---- stage C: scan, blk-pairs x 16 state dims ----
            ygs = {}
            for bp in range(2):
                ys = [pyp.tile([128, SC], F32, tag=f"y{i}", name=f"y{i}")
                      for i in range(2)]
                for n in range(DS):
                    bsb = bcp.tile([128, SC], BF16, tag="bsb", name="bsb")
                    csb = bcp.tile([128, SC], BF16, tag="csb", name="csb")
                    for hf in range(2):
                        bps = pbc.tile([128, TC], F32, tag="bcB", name="bps")
                        nc.tensor.matmul(bps[:], selb_sb[:, n * 128:(n + 1) * 128],
                                         dbcrs[hf][:], start=True, stop=True)
                        nc.scalar.copy(bsb[:, hf * TC:(hf + 1) * TC], bps[:])
                        cps = pbc.tile([128, TC], F32, tag="bcC", name="cps")
                        nc.tensor.matmul(cps[:], selc_sb[:, n * 128:(n + 1) * 128],
                                         dbcrs[hf][:], start=True, stop=True)
                        nc.scalar.copy(csb[:, hf * TC:(hf + 1) * TC], cps[:])
                    for i in range(2):
                        blk = bp * 2 + i
                        col = blk * DS + n
                        da = scp.tile([128, SC], F32, tag="da")
                        nc.scalar.activation(da[:], delta[:, blk * SC:(blk + 1) * SC],
                                             AF.Exp, scale=a_sb[:, col:col + 1])
                        w2 = scp.tile([128, SC], BF16, tag="w2")
                        nc.vector.tensor_tensor(w2[:], du[:, blk * SC:(blk + 1) * SC],
                                                bsb[:], OP.mult)
                        h = scp.tile([128, SC], BF16, tag="h")
                        nc.vector.tensor_tensor_scan(h[:], da[:], w2[:],
                                                     state[:, col:col + 1],
                                                     OP.mult, OP.add)
                        if tp < NTP - 1:
                            nc.scalar.copy(state[:, col:col + 1], h[:, SC - 1:SC])
                        p = scp.tile([128, SC], BF16, tag="p")
                        nc.vector.tensor_tensor(p[:], h[:], csb[:], OP.mult)
                        for hf in range(2):
                            nc.tensor.matmul(ys[i][:, hf * TC:(hf + 1) * TC],
                                             idenb_sb[:], p[:, hf * TC:(hf + 1) * TC],
                                             start=(n == 0), stop=(n == DS - 1))
                # ---- stage D for this blk-pair ----
                for i in range(2):
                    blk = bp * 2 + i
                    for hf in range(2):
                        yf = gp.tile([128, TC], F32, tag="yf")
                        nc.vector.scalar_tensor_tensor(
                            yf[:], xcl[:, blk * SC + hf * TC:blk * SC + hf * TC + TC],
                            dvec_sb[:, blk:blk + 1], ys[i][:, hf * TC:(hf + 1) * TC],
                            OP.mult, OP.add)
                        yg = ygp.tile([128, TC], F32R, tag="yg", name="yg")
                        nc.vector.tensor_mul(
                            yg[:], yf[:],
                            zsil[:, blk * SC + hf * TC:blk * SC + hf * TC + TC])
                        ygs[(blk, hf)] = yg

            # ---- stage E: out_proj partials ----
            for hf in range(2):
                t = tp * 2 + hf
                for ob in range(NOB):
                    ps = pm.tile([128, TC], F32, tag="mm", name="pso")
                    for blk in range(NBLK):
                        nc.tensor.matmul(
                            ps[:],
                            wout_sb[:, blk * DM + ob * 128:blk * DM + ob * 128 + 128],
                            ygs[(blk, hf)][:],
                            start=(blk == 0), stop=(blk == NBLK - 1))
                    osb = osp.tile([128, TC], F32, tag="osb")
                    nc.scalar.copy(osb[:], ps[:])
                    nc.sync.dma_start(outp[:, ob * T + t * TC:ob * T + t * TC + TC],
                                      osb[:])


_NC_CACHE = None


def _get_program():
    global _NC_CACHE
    if _NC_CACHE is None:
        _NC_CACHE = _build_program()
    return _NC_CACHE


def _prep_core_inputs(x, params, g, b, dh):
    f32 = np.float32
    in_w = params["in_w"]; conv_w = params["conv_w"]; conv_b = params["conv_b"]
    xproj_w = params["xproj_w"]; dt_w = params["dt_w"]; dt_b = params["dt_b"]
    A_log = params["A_log"]; Dp = params["D"]; out_w = params["out_w"]

    if g == 0:
        xd = x[b, :, :DM]
    else:
        xd = x[b, ::-1, DM:]
    xd = np.ascontiguousarray(xd, dtype=f32)          # [T, DM]
    xt_pad = np.concatenate([np.zeros((3, DM), f32), xd], axis=0).T  # [DM, T+3]
    xt = round_f32r(
        xt_pad.reshape(NKC, 128, T + 3).transpose(1, 0, 2).reshape(128, NKC * (T + 3)))

    dloc = slice(dh * DL, (dh + 1) * DL)
    in_w_loc = in_w[dloc]                              # [DL, DM] (xh rows)
    conv_w_loc = conv_w[dloc]                          # [DL, KW]
    conv_b_loc = conv_b[dloc]

    wcin_cols = []
    for k in range(KW):
        mk = (in_w_loc * conv_w_loc[:, k:k + 1]).T     # [DM, DL]
        mk = mk.reshape(NKC, 128, DL)
        for kc in range(NKC):
            wcin_cols.append(mk[kc])
    wcin = round_f32r(np.concatenate(wcin_cols, axis=1).astype(f32))

    wz_m = in_w[DI + dh * DL: DI + (dh + 1) * DL].T    # [DM, DL]
    wz = round_f32r(
        wz_m.reshape(NKC, 128, DL).transpose(1, 0, 2).reshape(128, NKC * DL).astype(f32))

    bconv = np.ascontiguousarray(conv_b_loc.reshape(NBLK, 128).T.astype(f32))

    wxp_m = xproj_w[:, dloc].T                         # [DL, 64]
    wxp = np.ascontiguousarray(
        wxp_m.reshape(NBLK, 128, 64).transpose(1, 0, 2).reshape(128, NBLK * 64).astype(f32))

    wdt = round_f32r(dt_w[dloc].T.astype(f32))         # [32, DL]
    bdt = np.ascontiguousarray(dt_b[dloc].reshape(NBLK, 128).T.astype(f32))
    alog = np.ascontiguousarray(
        A_log[dloc].reshape(NBLK, 128, DS).transpose(1, 0, 2).reshape(128, NBLK * DS).astype(f32))
    dvec = np.ascontiguousarray(Dp[dloc].reshape(NBLK, 128).T.astype(f32))
    wout_m = out_w[:, dloc].T                          # [DL, DM]
    wout = round_f32r(
        wout_m.reshape(NBLK, 128, DM).transpose(1, 0, 2).reshape(128, NBLK * DM).astype(f32))

    idenb = np.eye(128).astype(ml_dtypes.bfloat16)
    selb = np.zeros((64, DS * 128), f32)
    selc = np.zeros((64, DS * 128), f32)
    for n in range(DS):
        selb[32 + n, n * 128:(n + 1) * 128] = 1.0
        selc[48 + n, n * 128:(n + 1) * 128] = 1.0

    return {"xt": xt, "wcin": wcin, "wz": wz, "bconv": bconv, "wxp": wxp,
            "wdt": wdt, "bdt": bdt, "alog": alog, "dvec": dvec, "wout": wout,
            "idenb": idenb, "selb": selb, "selc": selc}


def kernel(x,
           in_w1, conv_w1, conv_b1, xproj_w1, dt_w1, dt_b1, A_log1, D1, out_w1,
           in_w2, conv_w2, conv_b2, xproj_w2, dt_w2, dt_b2, A_log2, D2, out_w2):
    global LAST_EXEC_NS, LAST_RESULTS
    x = np.asarray(x, np.float32)
    p1 = dict(in_w=in_w1, conv_w=conv_w1, conv_b=conv_b1, xproj_w=xproj_w1,
              dt_w=dt_w1, dt_b=dt_b1, A_log=A_log1, D=D1, out_w=out_w1)
    p2 = dict(in_w=in_w2, conv_w=conv_w2, conv_b=conv_b2, xproj_w=xproj_w2,
              dt_w=dt_w2, dt_b=dt_b2, A_log=A_log2, D=D2, out_w=out_w2)
    p1 = {k: np.asarray(v, np.float32) for k, v in p1.items()}
    p2 = {k: np.asarray(v, np.float32) for k, v in p2.items()}

    in_maps = []
    for g, params in ((0, p1), (1, p2)):
        for b in range(2):
            for dh in range(2):
                in_maps.append(_prep_core_inputs(x, params, g, b, dh))

    nc = _get_program()
    trace = os.environ.get("BASS_KERNEL_TRACE", "0") == "1"
    try:
        res = run_bass_kernel_spmd(nc, in_maps, list(range(8)), trace=trace)
    except (ImportError, ModuleNotFoundError):
        res = run_bass_kernel_spmd(nc, in_maps, list(range(8)), trace=False)
    LAST_EXEC_NS = res.exec_time_ns
    LAST_RESULTS = res

    hidden = np.empty((2, T, 2 * DM), np.float32)
    for g in range(2):
        for b in range(2):
            c0 = g * 4 + b * 2
            part = res.results[c0]["outp"] + res.results[c0 + 1]["outp"]
            part = part.reshape(128, NOB, T).transpose(1, 0, 2).reshape(DM, T)
            hidden[b, :, g * DM:(g + 1) * DM] = part.T
    return hidden, x


# revision 8
# speedup vs baseline: 1.1636x; 1.1636x over previous
"""Bi-directional Mamba block (concat variant) on 8 Trainium2 NeuronCores.

Sharding: core = (direction g in {0,1}) x (batch b in {0,1}) x (d_inner half dh in {0,1}).
Each core runs one direction's Mamba for one batch element over its local 512 of
the 1024 d_inner channels.  The causal depthwise conv is folded into the input
projection as 4 time-shifted matmuls (host pre-merges conv_w into in_w).  The
x-projection contracts over all of d_inner, so the two cores of a (g,b) pair
AllReduce their [64, 512] partial per time chunk.  out_proj partials (contraction
over local channels) are summed on the host during unsharding.

Device layout is [channel-partition, time-free].  The SSM scan uses the hardware
tensor_tensor_scan (VectorE) over 1024-wide time spans: per (d-block of 128,
state n of 16), ScalarE computes dA = exp(delta * A[:,n]) with A as per-partition
activation scale, VectorE forms dBu = (delta*xc) * B_n and C*h in bf16 (2x DVE
mode), and the 16 state planes are summed by PE identity-matmuls into PSUM.
B_n/C_n rows are broadcast across partitions with selector matmuls on the PE.
"""

import os
import sys

sys.path.insert(0, "/opt/trn_rl_repo")

import numpy as np
import ml_dtypes
import concourse.bacc as bacc
import concourse.mybir as mybir
import concourse.tile as tile
from concourse.bass_utils import run_bass_kernel_spmd

F32 = mybir.dt.float32
F32R = mybir.dt.float32r
BF16 = mybir.dt.bfloat16
AF = mybir.ActivationFunctionType
OP = mybir.AluOpType

T = 2048          # sequence length
DM = 512          # per-direction d_model
DI = 1024         # full d_inner
DL = 512          # local d_inner channels per core
DS = 16           # d_state
RK = 32           # dt_rank
KW = 4            # d_conv
TC = 512          # time chunk (stage B / PSUM granularity)
SC = 1024         # scan span (two time chunks)
NTP = T // SC     # 2 tc-pairs
NKC = DM // 128   # 4 contraction chunks for in_proj
NBLK = DL // 128  # 4 local channel blocks
NOB = DM // 128   # 4 output blocks

GROUPS = [[0, 1], [2, 3], [4, 5], [6, 7]]

LAST_EXEC_NS = None
LAST_RESULTS = None


def round_f32r(x):
    """Round fp32 to fp32r (11-bit mantissa, round-to-nearest-even)."""
    u = np.ascontiguousarray(x, np.float32).view(np.uint32)
    lsb = (u >> 12) & np.uint32(1)
    ur = (u + np.uint32(0x7FF) + lsb) & np.uint32(0xFFFFF000)
    return ur.view(np.float32)


def _build_program():
    nc = bacc.Bacc("TRN2", target_bir_lowering=False, debug=False, num_devices=8)

    d = lambda name, shape: nc.dram_tensor(name, shape, F32, kind="ExternalInput").ap()
    dr = lambda name, shape: nc.dram_tensor(name, shape, F32R, kind="ExternalInput").ap()
    xt = dr("xt", [128, NKC * (T + 3)])         # x dir-half, transposed, 3-col zero pad, kc-major
    wcin = dr("wcin", [128, KW * NKC * DL])     # conv-fused in_proj lhsT, (k,kc)-major
    wz = dr("wz", [128, NKC * DL])              # z in_proj lhsT, kc-major
    bconv = d("bconv", [128, NBLK])
    wxp = d("wxp", [128, NBLK * 64])            # xproj lhsT (local), kc-major; fp32 mm
    wdt = dr("wdt", [32, DL])                   # dt_proj lhsT
    bdt = d("bdt", [128, NBLK])
    alog = d("alog", [128, NBLK * DS])
    dvec = d("dvec", [128, NBLK])
    wout = dr("wout", [128, NBLK * DM])         # out_proj lhsT, dblk-major
    idenb = nc.dram_tensor("idenb", [128, 128], BF16, kind="ExternalInput").ap()
    outp = nc.dram_tensor("outp", [128, NOB * T], F32, kind="ExternalOutput").ap()

    with tile.TileContext(nc) as tc_:
        _body(tc_, nc, xt, wcin, wz, bconv, wxp, wdt, bdt, alog, dvec, wout,
              idenb, outp)
    nc.compile()
    return nc


def _body(tc_, nc, xt, wcin, wz, bconv, wxp, wdt, bdt, alog, dvec, wout,
          idenb, outp):
    from contextlib import ExitStack
    ctx = ExitStack()
    with ctx:
        wp = ctx.enter_context(tc_.tile_pool(name="wp", bufs=1))
        xtp = ctx.enter_context(tc_.tile_pool(name="xtp", bufs=5))
        wcp = ctx.enter_context(tc_.tile_pool(name="wcp", bufs=4))
        seq = ctx.enter_context(tc_.tile_pool(name="seq", bufs=2))
        sq1 = ctx.enter_context(tc_.tile_pool(name="sq1", bufs=1))
        scp = ctx.enter_context(tc_.tile_pool(name="scp", bufs=2))
        bcp = ctx.enter_context(tc_.tile_pool(name="bcp", bufs=2))
        stp = ctx.enter_context(tc_.tile_pool(name="stp", bufs=4))
        gp = ctx.enter_context(tc_.tile_pool(name="gp", bufs=2))
        ygp = ctx.enter_context(tc_.tile_pool(name="ygp", bufs=8))
        osp = ctx.enter_context(tc_.tile_pool(name="osp", bufs=2))
        drp = ctx.enter_context(tc_.tile_pool(name="drp", bufs=2, space="DRAM"))
        pm = ctx.enter_context(tc_.tile_pool(name="pm", bufs=4, space="PSUM"))
        pyp = ctx.enter_context(tc_.tile_pool(name="pyp", bufs=1, space="PSUM"))

        # ---- persistent weights ----
        def wtile(name, shape, src, dt_=F32):
            t_ = wp.tile(shape, dt_, tag=name, name=name)
            nc.sync.dma_start(t_[:], src[:])
            return t_

        wz_sb = wtile("wz", [128, NKC * DL], wz, F32R)
        wxp_sb = wtile("wxp", [128, NBLK * 64], wxp)
        wdt_sb = wtile("wdt", [32, DL], wdt, F32R)
        bdt_sb = wtile("bdt", [128, NBLK], bdt)
        bconv_sb = wtile("bconv", [128, NBLK], bconv)
        alog_sb = wtile("alog", [128, NBLK * DS], alog)
        dvec_sb = wtile("dvec", [128, NBLK], dvec)
        wout_sb = wtile("wout", [128, NBLK * DM], wout, F32R)
        idenb_sb = wtile("idenb", [128, 128], idenb, BF16)

        # A = -exp(A_log)
        a_tmp = wp.tile([128, NBLK * DS], F32, tag="a_tmp")
        nc.scalar.activation(a_tmp[:], alog_sb[:], AF.Exp)
        a_sb = wp.tile([128, NBLK * DS], F32, tag="a_sb")
        nc.vector.tensor_scalar_mul(a_sb[:], a_tmp[:], -1.0)

        # scan state [128, blk*16+n], init 0
        state = wp.tile([128, NBLK * DS], F32, tag="state")
        nc.vector.memset(state[:], 0.0)

        for tp in range(NTP):
            dbcbf = bcp.tile([64, SC], BF16, tag="dbcbf", bufs=2, name="dbcbf")
            xcl = sq1.tile([128, NBLK * SC], F32, tag="xcl")
            zsil = sq1.tile([128, NBLK * SC], F32, tag="zsil")
            delta = seq.tile([128, NBLK * SC], F32, tag="delta")
            dbcrs = []
            for hf in range(2):
                t = tp * 2 + hf
                # ---- stage B ----
                xts = []
                for kc in range(NKC):
                    xtile = xtp.tile([128, TC + 3], F32R, tag="xts", name="xtile")
                    nc.sync.dma_start(xtile[:], xt[:, kc * (T + 3) + t * TC:
                                                   kc * (T + 3) + t * TC + TC + 3])
                    xts.append(xtile)

                # conv-fused in_proj, single-pass weight stream, 4 psum tiles
                pss = [pm.tile([128, TC], F32, tag="mm", name="psin")
                       for _ in range(NBLK)]
                for k in range(KW):
                    for kc in range(NKC):
                        wtl = wcp.tile([128, DL], F32R, tag="wcin", name="wtl")
                        nc.sync.dma_start(
                            wtl[:], wcin[:, (k * NKC + kc) * DL:
                                         (k * NKC + kc) * DL + DL])
                        for mb in range(NBLK):
                            nc.tensor.matmul(
                                pss[mb][:], wtl[:, mb * 128:(mb + 1) * 128],
                                xts[kc][:, k:k + TC],
                                start=(k == 0 and kc == 0),
                                stop=(k == KW - 1 and kc == NKC - 1))
                for mb in range(NBLK):
                    nc.scalar.activation(
                        xcl[:, mb * SC + hf * TC:mb * SC + hf * TC + TC],
                        pss[mb][:], AF.Silu, bias=bconv_sb[:, mb:mb + 1])

                # xproj partial (local half) -> AllReduce across the (g,b) pair
                psd = pm.tile([64, TC], F32, tag="mm", name="psd")
                for mb in range(NBLK):
                    nc.tensor.matmul(
                        psd[:], wxp_sb[:, mb * 64:(mb + 1) * 64],
                        xcl[:, mb * SC + hf * TC:mb * SC + hf * TC + TC],
                        start=(mb == 0), stop=(mb == NBLK - 1))
                dbp = gp.tile([64, TC], F32, tag="dbp", bufs=1)
                nc.scalar.copy(dbp[:], psd[:])
                dbi = drp.tile([64, TC], F32, tag="dbi")
                dbo = drp.tile([64, TC], F32, tag="dbo")
                nc.sync.dma_start(dbi[:], dbp[:])
                nc.gpsimd.collective_compute(
                    "AllReduce", OP.add, replica_groups=GROUPS,
                    ins=[dbi.opt()], outs=[dbo.opt()])
                dbc = gp.tile([64, TC], F32, tag="dbc", bufs=1)
                nc.sync.dma_start(dbc[:], dbo[:])
                dbcr = gp.tile([64, TC], F32R, tag="dbcr")
                nc.scalar.copy(dbcr[:], dbc[:])
                dbcrs.append(dbcr)
                nc.scalar.copy(dbcbf[:, hf * TC:(hf + 1) * TC], dbc[:])

                # z branch (local half only)
                for zb in range(NBLK):
                    ps = pm.tile([128, TC], F32, tag="mm", name="psz")
                    for kc in range(NKC):
                        nc.tensor.matmul(
                            ps[:],
                            wz_sb[:, kc * DL + zb * 128:kc * DL + zb * 128 + 128],
                            xts[kc][:, 3:3 + TC],
                            start=(kc == 0), stop=(kc == NKC - 1))
                    nc.scalar.activation(zsil[:, zb * SC + hf * TC:
                                               zb * SC + hf * TC + TC], ps[:], AF.Silu)

                # delta = softplus(dt_proj + dt_b) = ln(1 + e^x), x clamped at 80
                for blk in range(NBLK):
                    ps = pm.tile([128, TC], F32, tag="mm", name="psdt")
                    nc.tensor.matmul(
                        ps[:], wdt_sb[:, blk * 128:(blk + 1) * 128],
                        dbcr[0:32, :], start=True, stop=True)
                    spt = scp.tile([128, TC], F32, tag="sptmp")
                    nc.vector.tensor_scalar(spt[:], ps[:], bdt_sb[:, blk:blk + 1],
                                            80.0, OP.add, OP.min)
                    spe = scp.tile([128, TC], F32, tag="spexp")
                    nc.scalar.activation(spe[:], spt[:], AF.Exp)
                    nc.scalar.activation(delta[:, blk * SC + hf * TC:
                                               blk * SC + hf * TC + TC],
                                         spe[:], AF.Ln, bias=1.0)

            # du = delta * xc_local (bf16 for the 2x DVE path)
            du = seq.tile([128, NBLK * SC], BF16, tag="du")
            for blk in range(NBLK):
                nc.vector.tensor_mul(du[:, blk * SC:(blk + 1) * SC],
                                     delta[:, blk * SC:(blk + 1) * SC],
                                     xcl[:, blk * SC:(blk + 1) * SC])

            # ---- stage C: scan, blk-pairs x 16 state dims ----
            ygs = {}
            for bp in range(2):
                ys = [pyp.tile([128, SC], F32, tag=f"y{i}", name=f"y{i}")
                      for i in range(2)]
                for n in range(DS):
                    stb = stp.tile([1, SC], BF16, tag="stb", name="stb")
                    nc.sync.dma_start(stb[:], dbcbf[32 + n:33 + n, :])
                    bsb = bcp.tile([128, SC], BF16, tag="bsb", name="bsb")
                    nc.gpsimd.partition_broadcast(bsb[:], stb[:])
                    stc = stp.tile([1, SC], BF16, tag="stc", name="stc")
                    nc.sync.dma_start(stc[:], dbcbf[48 + n:49 + n, :])
                    csb = bcp.tile([128, SC], BF16, tag="csb", name="csb")
                    nc.gpsimd.partition_broadcast(csb[:], stc[:])
                    for i in range(2):
                        blk = bp * 2 + i
                        col = blk * DS + n
                        da = scp.tile([128, SC], F32, tag="da")
                        nc.scalar.activation(da[:], delta[:, blk * SC:(blk + 1) * SC],
                                             AF.Exp, scale=a_sb[:, col:col + 1])
                        w2 = scp.tile([128, SC], BF16, tag="w2")
                        nc.vector.tensor_tensor(w2[:], du[:, blk * SC:(blk + 1) * SC],
                                                bsb[:], OP.mult)
                        h = scp.tile([128, SC], BF16, tag="h")
                        nc.vector.tensor_tensor_scan(h[:], da[:], w2[:],
                                                     state[:, col:col + 1],
                                                     OP.mult, OP.add)
                        if tp < NTP - 1:
                            nc.scalar.copy(state[:, col:col + 1], h[:, SC - 1:SC])
                        p = scp.tile([128, SC], BF16, tag="p")
                        nc.vector.tensor_tensor(p[:], h[:], csb[:], OP.mult)
                        for hf in range(2):
                            nc.tensor.matmul(ys[i][:, hf * TC:(hf + 1) * TC],
                                             idenb_sb[:], p[:, hf * TC:(hf + 1) * TC],
                                             start=(n == 0), stop=(n == DS - 1))
                # ---- stage D for this blk-pair ----
                for i in range(2):
                    blk = bp * 2 + i
                    for hf in range(2):
                        yf = gp.tile([128, TC], F32, tag="yf")
                        nc.vector.scalar_tensor_tensor(
                            yf[:], xcl[:, blk * SC + hf * TC:blk * SC + hf * TC + TC],
                            dvec_sb[:, blk:blk + 1], ys[i][:, hf * TC:(hf + 1) * TC],
                            OP.mult, OP.add)
                        yg = ygp.tile([128, TC], F32R, tag="yg", name="yg")
                        nc.vector.tensor_mul(
                            yg[:], yf[:],
                            zsil[:, blk * SC + hf * TC:blk * SC + hf * TC + TC])
                        ygs[(blk, hf)] = yg

            # ---- stage E: out_proj partials ----
            for hf in range(2):
                t = tp * 2 + hf
                for ob in range(NOB):
                    ps = pm.tile([128, TC], F32, tag="mm", name="pso")
                    for blk in range(NBLK):
                        nc.tensor.matmul(
                            ps[:],
                            wout_sb[:, blk * DM + ob * 128:blk * DM + ob * 128 + 128],
                            ygs[(blk, hf)][:],
                            start=(blk == 0), stop=(blk == NBLK - 1))
                    osb = osp.tile([128, TC], F32, tag="osb")
                    nc.scalar.copy(osb[:], ps[:])
                    nc.sync.dma_start(outp[:, ob * T + t * TC:ob * T + t * TC + TC],
                                      osb[:])


_NC_CACHE = None


def _get_program():
    global _NC_CACHE
    if _NC_CACHE is None:
        _NC_CACHE = _build_program()
    return _NC_CACHE


def _prep_core_inputs(x, params, g, b, dh):
    f32 = np.float32
    in_w = params["in_w"]; conv_w = params["conv_w"]; conv_b = params["conv_b"]
    xproj_w = params["xproj_w"]; dt_w = params["dt_w"]; dt_b = params["dt_b"]
    A_log = params["A_log"]; Dp = params["D"]; out_w = params["out_w"]

    if g == 0:
        xd = x[b, :, :DM]
    else:
        xd = x[b, ::-1, DM:]
    xd = np.ascontiguousarray(xd, dtype=f32)          # [T, DM]
    xt_pad = np.concatenate([np.zeros((3, DM), f32), xd], axis=0).T  # [DM, T+3]
    xt = round_f32r(
        xt_pad.reshape(NKC, 128, T + 3).transpose(1, 0, 2).reshape(128, NKC * (T + 3)))

    dloc = slice(dh * DL, (dh + 1) * DL)
    in_w_loc = in_w[dloc]                              # [DL, DM] (xh rows)
    conv_w_loc = conv_w[dloc]                          # [DL, KW]
    conv_b_loc = conv_b[dloc]

    wcin_cols = []
    for k in range(KW):
        mk = (in_w_loc * conv_w_loc[:, k:k + 1]).T     # [DM, DL]
        mk = mk.reshape(NKC, 128, DL)
        for kc in range(NKC):
            wcin_cols.append(mk[kc])
    wcin = round_f32r(np.concatenate(wcin_cols, axis=1).astype(f32))

    wz_m = in_w[DI + dh * DL: DI + (dh + 1) * DL].T    # [DM, DL]
    wz = round_f32r(
        wz_m.reshape(NKC, 128, DL).transpose(1, 0, 2).reshape(128, NKC * DL).astype(f32))

    bconv = np.ascontiguousarray(conv_b_loc.reshape(NBLK, 128).T.astype(f32))

    wxp_m = xproj_w[:, dloc].T                         # [DL, 64]
    wxp = np.ascontiguousarray(
        wxp_m.reshape(NBLK, 128, 64).transpose(1, 0, 2).reshape(128, NBLK * 64).astype(f32))

    wdt = round_f32r(dt_w[dloc].T.astype(f32))         # [32, DL]
    bdt = np.ascontiguousarray(dt_b[dloc].reshape(NBLK, 128).T.astype(f32))
    alog = np.ascontiguousarray(
        A_log[dloc].reshape(NBLK, 128, DS).transpose(1, 0, 2).reshape(128, NBLK * DS).astype(f32))
    dvec = np.ascontiguousarray(Dp[dloc].reshape(NBLK, 128).T.astype(f32))
    wout_m = out_w[:, dloc].T                          # [DL, DM]
    wout = round_f32r(
        wout_m.reshape(NBLK, 128, DM).transpose(1, 0, 2).reshape(128, NBLK * DM).astype(f32))

    idenb = np.eye(128).astype(ml_dtypes.bfloat16)

    return {"xt": xt, "wcin": wcin, "wz": wz, "bconv": bconv, "wxp": wxp,
            "wdt": wdt, "bdt": bdt, "alog": alog, "dvec": dvec, "wout": wout,
            "idenb": idenb}


def kernel(x,
           in_w1, conv_w1, conv_b1, xproj_w1, dt_w1, dt_b1, A_log1, D1, out_w1,
           in_w2, conv_w2, conv_b2, xproj_w2, dt_w2, dt_b2, A_log2, D2, out_w2):
    global LAST_EXEC_NS, LAST_RESULTS
    x = np.asarray(x, np.float32)
    p1 = dict(in_w=in_w1, conv_w=conv_w1, conv_b=conv_b1, xproj_w=xproj_w1,
              dt_w=dt_w1, dt_b=dt_b1, A_log=A_log1, D=D1, out_w=out_w1)
    p2 = dict(in_w=in_w2, conv_w=conv_w2, conv_b=conv_b2, xproj_w=xproj_w2,
              dt_w=dt_w2, dt_b=dt_b2, A_log=A_log2, D=D2, out_w=out_w2)
    p1 = {k: np.asarray(v, np.float32) for k, v in p1.items()}
    p2 = {k: np.asarray(v, np.float32) for k, v in p2.items()}

    in_maps = []
    for g, params in ((0, p1), (1, p2)):
        for b in range(2):
            for dh in range(2):
                in_maps.append(_prep_core_inputs(x, params, g, b, dh))

    nc = _get_program()
    trace = os.environ.get("BASS_KERNEL_TRACE", "0") == "1"
    try:
        res = run_bass_kernel_spmd(nc, in_maps, list(range(8)), trace=trace)
    except (ImportError, ModuleNotFoundError):
        res = run_bass_kernel_spmd(nc, in_maps, list(range(8)), trace=False)
    LAST_EXEC_NS = res.exec_time_ns
    LAST_RESULTS = res

    hidden = np.empty((2, T, 2 * DM), np.float32)
    for g in range(2):
        for b in range(2):
            c0 = g * 4 + b * 2
            part = res.results[c0]["outp"] + res.results[c0 + 1]["outp"]
            part = part.reshape(128, NOB, T).transpose(1, 0, 2).reshape(DM, T)
            hidden[b, :, g * DM:(g + 1) * DM] = part.T
    return hidden, x
